# revision 9
# baseline (speedup 1.0000x reference)
"""Disentangled spatial attention on 8 TRN2 NeuronCores.

Sharding: (batch b in 0..3) x (query-half qh in 0..1) -> 8 cores, no
collectives.  Each core computes the full attention output rows for its
1024 query tokens of its batch, over all 16 heads and all 2048 kv tokens.
Host splits/scales weights, reorders tokens (q-half first), casts to bf16,
and concatenates per-core outputs back into the full [B,S,2,E] tensor
(spatial stream is the untouched input passthrough).

Math (general lambda):
  qcomb_h = [qt_h ; qs_h]              (128 dims on partitions)
  kcomb_h = [kt_h + lts*ks_h ; lst*kt_h + lss*ks_h]
  scores  = qcomb_h . kcomb_h          (one K=128 matmul per tile)
  softmax without max-subtraction (scores are O(5) bounded here),
  denominators via 64 all-ones columns appended to v (M=128 AV matmul
  yields y rows and 64 replicated denominator rows in one pass).
Odd heads use a partition-flipped layout ([qs;qt], [k2;k1], [ones|v]) so
every DVE op stays partition-aligned; the only cross-partition moves are
DMAs (kcomb spill through DRAM, denominator 64-row shift).
"""

import os
import sys
import math

import numpy as np

for _p in ("/opt/trn_rl_repo",):
    if os.path.isdir(_p) and _p not in sys.path:
        sys.path.insert(0, _p)

import ml_dtypes

import concourse.bass as bass
import concourse.bacc as bacc_mod
import concourse.mybir as mybir
import concourse.tile as tile
from concourse.bass_utils import run_bass_kernel_spmd

F32 = mybir.dt.float32
BF16 = mybir.dt.bfloat16
AF = mybir.ActivationFunctionType


def build_nc(S=2048, Sq=1024, E=1024, H=16, lst=1.0, rank1=True):
    """Build the per-core Bass program (SPMD across 8 cores).

    S: kv tokens per core; Sq: query tokens per core (first Sq tokens of xb);
    E: embed dim; H: heads; lst: lambda_st (baked for the rank-1 bottom-half
    scale); rank1: lambda_ss == lambda_ts * lambda_st.
    """
    D = E // H
    scale = 1.0 / math.sqrt(D)
    FC = E // 128   # feature chunks (contraction)
    TC = S // 128   # kv token chunks
    QC = Sq // 128  # query token chunks
    MT = E // 128   # output dim chunks (head pairs)
    NQ = [(n, min(512, Sq - n)) for n in range(0, Sq, 512)]
    NE = [(n, min(512, E - n)) for n in range(0, E, 512)]

    nc = bacc_mod.Bacc("TRN2", target_bir_lowering=False)
    xb = nc.dram_tensor("xb", [S, 2, E], BF16, kind="ExternalInput")
    wq = nc.dram_tensor("wq", [E, 2 * E], BF16, kind="ExternalInput")
    wk = nc.dram_tensor("wk", [E, 2 * E], BF16, kind="ExternalInput")
    if not rank1:
        wkB = nc.dram_tensor("wkB", [E, 2 * E], BF16, kind="ExternalInput")
    wv = nc.dram_tensor("wv", [E, E], BF16, kind="ExternalInput")
    wo = nc.dram_tensor("wo", [E, E], BF16, kind="ExternalInput")
    out = nc.dram_tensor("out", [Sq, E], F32, kind="ExternalOutput")
    kcomb_dram = nc.dram_tensor("kcomb_dram", [H, 128, S], BF16)

    with tile.TileContext(nc) as tc:
        with tc.tile_pool(name="pers", bufs=1) as pers:
            # persistent across phases
            qcomb = pers.tile([128, H, Sq], BF16)
            vt_sb = pers.tile([128, TC, E], BF16)
            yt_all = pers.tile([128, MT, Sq], BF16)

            # ---------------- phase 1: projections ----------------
            with tc.tile_pool(name="wpool", bufs=1) as wpool, \
                 tc.tile_pool(name="xtp", bufs=1) as xtp, \
                 tc.tile_pool(name="psum1", bufs=3, space="PSUM") as psum1, \
                 tc.tile_pool(name="stage", bufs=2) as stage:

                xtT = xtp.tile([128, FC, S], BF16)
                xsT = xtp.tile([128, FC, S], BF16)
                for st, xT in ((0, xtT), (1, xsT)):
                    # xbar: xT[p, di, t] = xb[t, st, di*128 + p]
                    nc.sync.dma_start(
                        out=xT[:, :, :], in_=xb[:, st, :], transpose=True)

                wq_sb = wpool.tile([128, FC, 2 * E], BF16, tag="w2e")
                nc.sync.dma_start(
                    out=wq_sb, in_=wq.rearrange("(c p) n -> p c n", p=128))

                # q projections -> qcomb ([qt;qs] even heads, [qs;qt] odd)
                # host pre-swapped the Wqs column pairs, so psum partition
                # ranges line up with qcomb partition ranges directly.
                for half, xT in ((0, xtT), (1, xsT)):
                    for m in range(MT):
                        ps = psum1.tile([128, Sq], F32, tag="ps1")
                        for f in range(FC):
                            for n0, nn in NQ:
                                nc.tensor.matmul(
                                    ps[:, n0:n0 + nn],
                                    lhsT=wq_sb[:, f, half * E + m * 128:
                                               half * E + (m + 1) * 128],
                                    rhs=xT[:, f, n0:n0 + nn],
                                    start=(f == 0), stop=(f == FC - 1))
                        if half == 0:   # qt: top of even head, bottom of odd
                            nc.vector.tensor_copy(
                                out=qcomb[0:64, 2 * m, :], in_=ps[0:64, :])
                            nc.vector.tensor_copy(
                                out=qcomb[64:128, 2 * m + 1, :], in_=ps[64:128, :])
                        else:           # qs (host-swapped): [qs_{2m+1}; qs_{2m}]
                            nc.vector.tensor_copy(
                                out=qcomb[0:64, 2 * m + 1, :], in_=ps[0:64, :])
                            nc.vector.tensor_copy(
                                out=qcomb[64:128, 2 * m, :], in_=ps[64:128, :])

                # v projection (natural [tok, dim]); x as stationary
                wv_sb = wpool.tile([128, FC, E], BF16, tag="w2e")
                nc.sync.dma_start(
                    out=wv_sb, in_=wv.rearrange("(c p) n -> p c n", p=128))
                for t in range(TC):
                    ps = psum1.tile([128, E], F32, tag="ps1")
                    for f in range(FC):
                        for n0, nn in NE:
                            nc.tensor.matmul(
                                ps[:, n0:n0 + nn],
                                lhsT=xtT[:, f, t * 128:(t + 1) * 128],
                                rhs=wv_sb[:, f, n0:n0 + nn],
                                start=(f == 0), stop=(f == FC - 1))
                    nc.vector.tensor_copy(out=vt_sb[:, t, :], in_=ps[:, :])

                # k projections -> kcomb staging -> DRAM spill
                def kproj(w_sb, mt):
                    # top half of kcomb (or bottom, from wkB): kt + c*ks
                    # staging rows: [head 2m piece (0:64); head 2m+1 piece]
                    st_t = stage.tile([128, S], BF16, tag="kst")
                    for nh in range(0, S, 1024):
                        nn = min(1024, S - nh)
                        pkt = psum1.tile([128, nn], F32, tag="ps1")
                        for f in range(FC):
                            for n0 in range(0, nn, 512):
                                ns = min(512, nn - n0)
                                nc.tensor.matmul(
                                    pkt[:, n0:n0 + ns],
                                    lhsT=w_sb[:, f, mt * 128:(mt + 1) * 128],
                                    rhs=xtT[:, f, nh + n0:nh + n0 + ns],
                                    start=(f == 0), stop=(f == FC - 1))
                        pks = psum1.tile([128, nn], F32, tag="ps1")
                        for f in range(FC):
                            for n0 in range(0, nn, 512):
                                ns = min(512, nn - n0)
                                nc.tensor.matmul(
                                    pks[:, n0:n0 + ns],
                                    lhsT=w_sb[:, f, E + mt * 128:
                                              E + (mt + 1) * 128],
                                    rhs=xsT[:, f, nh + n0:nh + n0 + ns],
                                    start=(f == 0), stop=(f == FC - 1))
                        nc.vector.tensor_copy(
                            out=st_t[:, nh:nh + nn], in_=pkt[:, :])
                        nc.vector.tensor_add(
                            out=st_t[:, nh:nh + nn],
                            in0=st_t[:, nh:nh + nn], in1=pks[:, :])
                    return st_t

                # kcomb_dram[2m]   = [top(0:64) ; bot(0:64)]
                # kcomb_dram[2m+1] = [bot(64:128) ; top(64:128)]  (flipped)
                wk_sb = wpool.tile([128, FC, 2 * E], BF16, tag="w2e")
                nc.sync.dma_start(
                    out=wk_sb, in_=wk.rearrange("(c p) n -> p c n", p=128))
                for m in range(MT):
                    st_top = kproj(wk_sb, m)
                    nc.sync.dma_start(
                        out=kcomb_dram[2 * m, 0:64, :], in_=st_top[0:64, :])
                    nc.sync.dma_start(
                        out=kcomb_dram[2 * m + 1, 64:128, :], in_=st_top[64:128, :])
                    if rank1:
                        st_bot = stage.tile([128, S], BF16, tag="kst")
                        nc.vector.tensor_scalar_mul(
                            out=st_bot[:, :], in0=st_top[:, :], scalar1=float(lst))
                        nc.sync.dma_start(
                            out=kcomb_dram[2 * m, 64:128, :], in_=st_bot[0:64, :])
                        nc.sync.dma_start(
                            out=kcomb_dram[2 * m + 1, 0:64, :], in_=st_bot[64:128, :])
                if not rank1:
                    wkB_sb = wpool.tile([128, FC, 2 * E], BF16, tag="w2e")
                    nc.sync.dma_start(
                        out=wkB_sb, in_=wkB.rearrange("(c p) n -> p c n", p=128))
                    for m in range(MT):
                        st_bot = kproj(wkB_sb, m)
                        nc.sync.dma_start(
                            out=kcomb_dram[2 * m, 64:128, :], in_=st_bot[0:64, :])
                        nc.sync.dma_start(
                            out=kcomb_dram[2 * m + 1, 0:64, :], in_=st_bot[64:128, :])

            # ---------------- phase 2: attention ----------------
            with tc.tile_pool(name="wo_p", bufs=1) as wo_p, \
                 tc.tile_pool(name="kp", bufs=2) as kp, \
                 tc.tile_pool(name="vp", bufs=2) as vp, \
                 tc.tile_pool(name="ptp", bufs=3) as ptp, \
                 tc.tile_pool(name="dnp", bufs=2) as dnp, \
                 tc.tile_pool(name="psA", bufs=2, space="PSUM") as psA, \
                 tc.tile_pool(name="psY", bufs=2, space="PSUM") as psY, \
                 tc.tile_pool(name="outp", bufs=2) as outp:

                wo_sb = wo_p.tile([128, MT, E], BF16)
                nc.sync.dma_start(
                    out=wo_sb, in_=wo.rearrange("(c p) n -> p c n", p=128))

                for h in range(H):
                    odd = h % 2
                    kc_h = kp.tile([128, S], BF16)
                    nc.sync.dma_start(out=kc_h, in_=kcomb_dram[h])
                    # vt_h columns: even head [v | ones], odd head [ones | v]
                    vt_h = vp.tile([128, TC, 128], BF16)
                    vcol, ocol = (0, 64) if not odd else (64, 0)
                    nc.vector.tensor_copy(
                        out=vt_h[:, :, vcol:vcol + 64],
                        in_=vt_sb[:, :, h * 64:(h + 1) * 64])
                    nc.vector.memset(vt_h[:, :, ocol:ocol + 64], 1.0)

                    yt = psY.tile([128, Sq], F32, tag="yt")
                    for kc in range(TC):
                        st_ = psA.tile([128, Sq], F32, tag="st")
                        for n0, nn in NQ:
                            nc.tensor.matmul(
                                st_[:, n0:n0 + nn],
                                lhsT=kc_h[:, kc * 128:(kc + 1) * 128],
                                rhs=qcomb[:, h, n0:n0 + nn],
                                start=True, stop=True)
                        pt = ptp.tile([128, Sq], BF16)
                        nc.scalar.activation(
                            out=pt[:, :], in_=st_[:, :], func=AF.Exp, scale=scale)
                        for n0, nn in NQ:
                            nc.tensor.matmul(
                                yt[:, n0:n0 + nn],
                                lhsT=vt_h[:, kc, :],
                                rhs=pt[:, n0:n0 + nn],
                                start=(kc == 0), stop=(kc == TC - 1))
                    # y rows at parts [64*odd, +64); denom replicas on the other half
                    ybase, dbase = (0, 64) if not odd else (64, 0)
                    dn = dnp.tile([128, Sq], F32)
                    nc.vector.reciprocal(
                        out=dn[dbase:dbase + 64, :], in_=yt[dbase:dbase + 64, :])
                    nc.sync.dma_start(
                        out=dn[ybase:ybase + 64, :], in_=dn[dbase:dbase + 64, :])
                    nc.vector.tensor_mul(
                        out=yt_all[ybase:ybase + 64, h // 2, :],
                        in0=yt[ybase:ybase + 64, :],
                        in1=dn[ybase:ybase + 64, :])

                # out projection: out[q, E] = sum_dc yt_all[:,dc,q].T @ wo[dc]
                for qt in range(QC):
                    ps = psA.tile([128, E], F32, tag="st")
                    for dc in range(MT):
                        for n0, nn in NE:
                            nc.tensor.matmul(
                                ps[:, n0:n0 + nn],
                                lhsT=yt_all[:, dc, qt * 128:(qt + 1) * 128],
                                rhs=wo_sb[:, dc, n0:n0 + nn],
                                start=(dc == 0), stop=(dc == MT - 1))
                    ob = outp.tile([128, E], F32)
                    nc.vector.tensor_copy(out=ob, in_=ps)
                    nc.sync.dma_start(
                        out=out[qt * 128:(qt + 1) * 128, :], in_=ob)
    nc.compile()
    return nc


# ---------------------------------------------------------------------------
# host side
# ---------------------------------------------------------------------------

N_CORES = 8
_prog_cache = {}
last_results = None  # BassKernelResults of the most recent kernel() call


def _ensure_ntff_hook():
    """Provide antenv.axon_hooks (NTFF profiling registry) if the image
    lacks it, so run_bass_kernel_spmd(trace=True) can capture profiles."""
    try:
        import antenv.axon_hooks  # noqa: F401
        return
    except ImportError:
        pass
    import contextlib
    import ctypes
    import types

    mod = types.ModuleType("antenv.axon_hooks")
    state = {"hook": None, "tried": False}

    def set_axon_ntff_profile_hook(hook):
        state["hook"] = hook

    def _install_default():
        so_path = os.environ.get("AXON_PJRT_SO", "/opt/axon/libaxon_pjrt.so")
        if not os.path.exists(so_path):
            return None
        lib = ctypes.CDLL(so_path)
        if not hasattr(lib, "axon_start_nrt_profile"):
            return None
        lib.axon_start_nrt_profile.argtypes = [
            ctypes.POINTER(ctypes.c_int64), ctypes.c_size_t]
        lib.axon_start_nrt_profile.restype = ctypes.c_int64
        lib.axon_stop_nrt_profile.argtypes = [ctypes.c_char_p]
        lib.axon_stop_nrt_profile.restype = ctypes.c_int64

        @contextlib.contextmanager
        def _hook(output_dir, device_ids):
            import jax
            jax.devices()
            if device_ids:
                ids = (ctypes.c_int64 * len(device_ids))(*device_ids)
                rc = lib.axon_start_nrt_profile(ids, len(device_ids))
            else:
                rc = lib.axon_start_nrt_profile(None, 0)
            if rc != 0:
                raise RuntimeError(f"axon_start_nrt_profile rc={rc}")
            try:
                yield
            finally:
                n = lib.axon_stop_nrt_profile(str(output_dir).encode())
                print(f"ntff profile: {n} file(s) -> {output_dir}",
                      file=sys.stderr)

        return _hook

    def get_axon_ntff_profile_hook():
        if state["hook"] is None and not state["tried"]:
            state["tried"] = True
            state["hook"] = _install_default()
        return state["hook"]

    mod.set_axon_ntff_profile_hook = set_axon_ntff_profile_hook
    mod.get_axon_ntff_profile_hook = get_axon_ntff_profile_hook
    sys.modules["antenv.axon_hooks"] = mod
    try:
        import antenv
        antenv.axon_hooks = mod
    except ImportError:
        pass


def _get_prog(S, Sq, E, H, lst, rank1):
    key = (S, Sq, E, H, float(lst), bool(rank1))
    if key not in _prog_cache:
        _prog_cache[key] = build_nc(S=S, Sq=Sq, E=E, H=H, lst=lst, rank1=rank1)
    return _prog_cache[key]


def _swap_head_pairs(w, D):
    # swap column blocks (2m, 2m+1) -> (2m+1, 2m), block width D
    c = w.shape[1]
    wr = w.reshape(w.shape[0], c // (2 * D), 2, D)
    return np.ascontiguousarray(wr[:, :, ::-1, :]).reshape(w.shape[0], c)


def _prep(x, Wt, Ws, Wo, lam_ts, lam_st, lam_ss):
    E = Wt.shape[0]
    H = 16 if E == 1024 else max(1, E // 64)
    D = E // H
    lts = float(np.asarray(lam_ts).reshape(-1)[0])
    lst = float(np.asarray(lam_st).reshape(-1)[0])
    lss = float(np.asarray(lam_ss).reshape(-1)[0])
    rank1 = abs(lss - lts * lst) <= 1e-6 * max(1.0, abs(lss))

    bf = ml_dtypes.bfloat16
    Wqt, Wkt, Wv = Wt[:, :E], Wt[:, E:2 * E], Wt[:, 2 * E:3 * E]
    Wqs, Wks = Ws[:, :E], Ws[:, E:2 * E]

    wq = np.concatenate([Wqt, _swap_head_pairs(Wqs, D)], axis=1).astype(bf)
    wk = np.concatenate([Wkt, lts * Wks], axis=1).astype(bf)
    wkB = None
    if not rank1:
        wkB = np.concatenate([lst * Wkt, lss * Wks], axis=1).astype(bf)
    weights = {
        "wq": np.ascontiguousarray(wq),
        "wk": np.ascontiguousarray(wk),
        "wv": np.ascontiguousarray(Wv.astype(bf)),
        "wo": np.ascontiguousarray(Wo.astype(bf)),
    }
    if wkB is not None:
        weights["wkB"] = np.ascontiguousarray(wkB)
    return weights, lts, lst, lss, rank1, H, D


def kernel(x, Wt, Ws, Wo, lam_ts, lam_st, lam_ss):
    x = np.asarray(x)
    B, S, _, E = x.shape
    Sq = S // 2
    weights, lts, lst, lss, rank1, H, D = _prep(
        x, np.asarray(Wt), np.asarray(Ws), np.asarray(Wo),
        lam_ts, lam_st, lam_ss)

    nc = _get_prog(S, Sq, E, H, lst, rank1)

    bf = ml_dtypes.bfloat16
    x_bf = x.astype(bf)
    in_maps = []
    for c in range(N_CORES):
        b, qh = c // 2, c % 2
        xb = x_bf[b]
        if qh == 1:
            xb = np.concatenate([xb[Sq:], xb[:Sq]], axis=0)
        m = {"xb": np.ascontiguousarray(xb)}
        m.update(weights)
        in_maps.append(m)

    trace = bool(int(os.environ.get("KERNEL_TRACE", "0")))
    if trace:
        _ensure_ntff_hook()
    res = run_bass_kernel_spmd(nc, in_maps, list(range(N_CORES)), trace=trace)
    global last_results
    last_results = res
    results = res.results if hasattr(res, "results") else res

    y = np.empty((B, S, E), np.float32)
    for c in range(N_CORES):
        b, qh = c // 2, c % 2
        y[b, qh * Sq:(qh + 1) * Sq] = results[c]["out"]
    return np.ascontiguousarray(
        np.stack([y, x[:, :, 1, :].astype(np.float32)], axis=2))
